# revision 16
# baseline (speedup 1.0000x reference)
"""Trainium2 Bass kernel for multi-head attention (B=4, N=2048, C=256, H=16).

Sharding: 8 cores, core -> (batch b = core//2, head-group g = core%2, 8 heads).
Each core computes its 8 heads' attention plus a partial output projection;
the host sums the two partials per batch and adds b_proj.

Per-core design (v2 — engine-balanced):
  The workload is transcendental-bound: exp() of 8*2048*2048 = 33.5M score
  elements per core.  ScalarE (ACT) does exp at 1 elem/lane/cycle @1.2GHz
  (~218us if it does everything), so the exp is SPLIT between ACT (table exp,
  bf16 out) and VectorE (DVE) using a Schraudolph-style bit-trick:
      bf16_bits(exp(s)) ~= int16(s * 128*log2e + 128*(127-0.0585))
  one fused tensor_scalar (mult+add, RNE float->int16) then a free bitcast to
  bf16 (rel err ~1.8% rms, cancels partially in softmax).  DVE_KT of the 16
  key-tiles route their second head-pair through DVE.

  PE work uses array packing (tile_position):
   - scores: 4 heads concurrently in 32x128 row-tile mode, K=16 at row groups
     32j..32j+16 (spread q/k layout, fp16), S^T [keys,512q] per head,
     4 separate PSUM banks (two [128,1024] 2-bank tiles).  ~420ns/pass
     (LDWEIGHTS-bound).
   - attn@v: 4 heads concurrently in 128x32 col-tile mode, bf16, vaug
     [128 keys, 32] per head (16 v-dims, col16 = ones for the softmax
     denominator, rest zero), accumulated over the 16 key tiles into ONE
     PSUM bank [4 heads x 32 rows, 512q].  ~250ns/pass.
  Normalize: sel-matmul broadcasts the denominator row over each head group,
  reciprocal_approx_fast + multiply on DVE; projection in fp16 (FWL).
"""

import numpy as np

import concourse.bass as bass
import concourse.mybir as mybir
import concourse.tile as tile
from concourse import bacc

F32 = mybir.dt.float32
F32R = mybir.dt.float32r
F16 = mybir.dt.float16
BF16 = mybir.dt.bfloat16
I16 = mybir.dt.int16
ALU = mybir.AluOpType
EXPF = mybir.ActivationFunctionType.Exp
IDENT = mybir.ActivationFunctionType.Identity

P = 128
B, N_FULL, C, H, D = 4, 2048, 256, 16, 16
CC = C // P  # channel tiles (2)
NCORES = 8

LOG2E = 1.4426950408889634
SCH_C1 = 128.0 * LOG2E
SCH_MAGIC = 0.0585
SCH_C0 = 128.0 * (127.0 - SCH_MAGIC)

# how many of the 16 key-tiles route the second head-pair's exp through the
# DVE bit-trick instead of ScalarE (0 = all-ACT fallback, 16 = max DVE)
DVE_KT = 15

_NC_CACHE: dict = {}
LAST_RESULT = None  # BassKernelResults of the most recent run (for test.py)
TIMING_REPS = 1  # >1 repeats the compute on-device (timing); output unchanged

# timing-only ablations (break correctness): "" | "notail" | "noattn"
import os
ABLATE = os.environ.get("KABLATE", "")


def build(n_tokens=N_FULL, dve_kt=DVE_KT, reps=1):
    N = n_tokens
    KT = N // P   # key tiles
    QC = 512      # q-chunk (psum bank = 512 fp32)
    NQ = N // QC
    NT = N // P   # token tiles

    dve_mask = [(kt * dve_kt) % KT < dve_kt for kt in range(KT)]

    nc = bacc.Bacc()
    x_d = nc.dram_tensor("x", [N, C], F32, kind="ExternalInput")
    wq_d = nc.dram_tensor("wq", [2, C, P], F16, kind="ExternalInput")
    wk_d = nc.dram_tensor("wk", [2, C, P], F16, kind="ExternalInput")
    wv_d = nc.dram_tensor("wv", [C, P], F16, kind="ExternalInput")
    bq_d = nc.dram_tensor("bq", [2, P], F32, kind="ExternalInput")
    bk_d = nc.dram_tensor("bk", [2, P], F32, kind="ExternalInput")
    bv_d = nc.dram_tensor("bv", [P], F32, kind="ExternalInput")
    wp_d = nc.dram_tensor("wp", [2, P, C], F16, kind="ExternalInput")
    sel_d = nc.dram_tensor("sel", [P, P], F32R, kind="ExternalInput")
    idn_d = nc.dram_tensor("idn", [P, P], F32, kind="ExternalInput")
    out_d = nc.dram_tensor("out", [N, C], F32, kind="ExternalOutput")

    with tile.TileContext(nc) as tc:
        with (
            tc.tile_pool(name="const", bufs=1) as const,
            tc.tile_pool(name="work", bufs=6) as work,
            tc.tile_pool(name="ps_big", bufs=3, space="PSUM") as ps_big,
            tc.tile_pool(name="ps_at", bufs=2, space="PSUM") as ps_at,
        ):
            # ---------------- loads (small consts first) ----------------
            idn_sb = const.tile([P, P], F32, name="idn_sb")
            nc.sync.dma_start(idn_sb[:], idn_d[:])
            wq_sb = const.tile([P, 2, CC, P], F16, name="wq_sb")
            nc.sync.dma_start(
                wq_sb[:], wq_d[:].rearrange("g (cc p) f -> p g cc f", p=P)
            )
            wk_sb = const.tile([P, 2, CC, P], F16, name="wk_sb")
            nc.sync.dma_start(
                wk_sb[:], wk_d[:].rearrange("g (cc p) f -> p g cc f", p=P)
            )
            wv_sb = const.tile([P, CC, P], F16, name="wv_sb")
            nc.sync.dma_start(
                wv_sb[:], wv_d[:].rearrange("(cc p) f -> p cc f", p=P)
            )
            bq_sb = const.tile([P, 2], F32, name="bq_sb")
            nc.sync.dma_start(bq_sb[:], bq_d[:].rearrange("g p -> p g"))
            bk_sb = const.tile([P, 2], F32, name="bk_sb")
            nc.sync.dma_start(bk_sb[:], bk_d[:].rearrange("g p -> p g"))
            bv_sb = const.tile([P, 1], F32, name="bv_sb")
            nc.sync.dma_start(bv_sb[:], bv_d[:].rearrange("(p o) -> p o", o=1))
            wp_sb = const.tile([P, 2, C], F16, name="wp_sb")
            nc.sync.dma_start(wp_sb[:], wp_d[:].rearrange("g p c -> p g c"))
            sel_sb = const.tile([P, P], F32R, name="sel_sb")
            nc.sync.dma_start(sel_sb[:], sel_d[:])
            x_sb = const.tile([P, NT, C], F32, name="x_sb")
            x_r = x_d[:].rearrange("(t p) c -> p t c", p=P)
            for tt in range(NT):
                nc.sync.dma_start(x_sb[:, tt, :], x_r[:, tt, :])

            from contextlib import nullcontext

            loop_ctx = tc.For_i(0, reps, 1) if reps > 1 else nullcontext()
            with loop_ctx:
                _build_body(
                    nc, tc, const, work, ps_big, ps_at,
                    N, KT, QC, NQ, NT, dve_mask,
                    x_sb, wq_sb, wk_sb, wv_sb, wp_sb, sel_sb, idn_sb,
                    bq_sb, bk_sb, bv_sb, out_d,
                )
    nc.finalize()
    return nc


def _build_body(
    nc, tc, const, work, ps_big, ps_at,
    N, KT, QC, NQ, NT, dve_mask,
    x_sb, wq_sb, wk_sb, wv_sb, wp_sb, sel_sb, idn_sb,
    bq_sb, bk_sb, bv_sb, out_d,
):
    xt = const.tile([P, CC, N], F16, name="xt")
    qt = const.tile([P, 2, N], F16, name="qt")
    kt_sb = const.tile([P, 2, N], F16, name="kt_sb")
    vaug = const.tile([P, KT, 8, 32], BF16, name="vaug")

    # vaug: zeros, ones column at 16
    nc.vector.memset(vaug[:], 0.0)
    nc.vector.memset(vaug[:, :, :, 16:17], 1.0)

    # ---------------- phase A: xT, qkv projections, vaug ----------------
    # Split into sub-loops so the PE stream isn't blocked by evac copies.
    for c in range(NQ):
        # x^T for this 512-token chunk: 8 PE transposes into one 2-bank tile
        tp = ps_big.tile([P, 1024], F32, tag="big", name="tp")
        for ti in range(4):
            tt = 4 * c + ti
            for cc in range(CC):
                nc.tensor.transpose(
                    tp[:, 128 * (2 * ti + cc) : 128 * (2 * ti + cc + 1)],
                    x_sb[:, tt, 128 * cc : 128 * (cc + 1)],
                    idn_sb[:],
                )
        nc.vector.tensor_copy(
            xt[:, :, QC * c : QC * (c + 1)].rearrange(
                "p cc (t f) -> p cc t f", t=4
            ),
            tp[:].rearrange("p (t cc f) -> p cc t f", t=4, cc=CC),
        )

    # projection order: all k first, then q chunk 0 (unblocks attention),
    # then v/vaug, then the remaining q chunks
    def emit_proj(w_ap, b_ap, dst_eng, c):
        dst, eng, vt = dst_eng
        psq = ps_at.tile([P, QC], F32, tag="at", name="psq")
        for cc in range(CC):
            nc.tensor.matmul(
                psq[:],
                w_ap[:, cc, :],
                xt[:, cc, QC * c : QC * (c + 1)],
                start=(cc == 0),
                stop=(cc == CC - 1),
            )
        if eng == "act":
            nc.scalar.activation(
                dst[:, QC * c : QC * (c + 1)], psq[:], IDENT, bias=b_ap
            )
        else:
            nc.vector.tensor_scalar_add(vt[:], psq[:], b_ap)

    for c in range(NQ):
        emit_proj(wk_sb[:, 0], bk_sb[:, 0:1], (kt_sb[:, 0], "act", None), c)
        emit_proj(wk_sb[:, 1], bk_sb[:, 1:2], (kt_sb[:, 1], "act", None), c)
    emit_proj(wq_sb[:, 0], bq_sb[:, 0:1], (qt[:, 0], "act", None), 0)
    emit_proj(wq_sb[:, 1], bq_sb[:, 1:2], (qt[:, 1], "act", None), 0)
    vts = []
    for c in range(NQ):
        vt = work.tile([P, QC], F32, tag="vt", name="vt")
        vts.append(vt)
        emit_proj(wv_sb[:], bv_sb[:, 0:1], (None, "dve", vt), c)
    for c in range(NQ):
        # v^T -> vaug for this chunk's 4 key tiles
        vt = vts[c]
        tpv = ps_big.tile([P, 1024], F32, tag="big", name="tpv")
        for ti in range(4):
            nc.tensor.transpose(
                tpv[:, 128 * ti : 128 * (ti + 1)],
                vt[:, 128 * ti : 128 * (ti + 1)],
                idn_sb[:],
            )
        nc.vector.tensor_copy(
            vaug[:, 4 * c : 4 * (c + 1), :, 0:16],
            tpv[:, 0:512].rearrange("p (t h d) -> p t h d", t=4, h=8),
        )
    for c in range(1, NQ):
        emit_proj(wq_sb[:, 0], bq_sb[:, 0:1], (qt[:, 0], "act", None), c)
        emit_proj(wq_sb[:, 1], bq_sb[:, 1:2], (qt[:, 1], "act", None), c)

    # ---------------- phase B: attention (software-pipelined) ----------------
    # Emission order controls each engine's in-order queue: per key-tile we
    # emit exp(kt), then the next key-tile's first scores pair (so the PE can
    # run it while exp(kt) executes), then attnv(kt), then the second scores
    # pair.  The normalize/projection tail of each (nn,g2) step is deferred
    # into the next step's kt=0 block so it doesn't block the next scores.
    steps = [(nn, g2) for nn in range(NQ) for g2 in range(2)]
    ot_n_map = {}
    out_r = out_d[:].rearrange("(t p) c -> p t c", p=P)

    def emit_scores(nn, g2, kt, pair, js):
        for j in js:
            tgt = pair[0] if j < 2 else pair[1]
            nc.tensor.matmul(
                tgt[:, QC * (j % 2) : QC * (j % 2 + 1)],
                kt_sb[32 * j : 32 * j + D, g2, P * kt : P * (kt + 1)],
                qt[32 * j : 32 * j + D, g2, QC * nn : QC * (nn + 1)],
                start=True,
                stop=True,
                tile_position=(32 * j, 0),
            )

    def alloc_sc():
        return (
            ps_big.tile([P, 1024], F32, tag="big", name="sc01"),
            ps_big.tile([P, 1024], F32, tag="big", name="sc23"),
        )

    def make_tail(at, ot_n, nn, g2):
        if ABLATE == "notail":
            def tail_ablate():
                ob = work.tile([P, QC], F32, tag="ots", name="obx")
                nc.vector.tensor_copy(ob[:], at[:])
                nc.sync.dma_start(out_r[:, nn * 2 + g2, :], ob[:, 0:C])
            return tail_ablate

        def tail():
            ot_sb = work.tile([P, QC], F32R, tag="ots", name="ot_sb")
            nc.vector.tensor_copy(ot_sb[:], at[:])
            bc = ps_big.tile([P, QC], F32, tag="big", name="bc")
            nc.tensor.matmul(bc[:], sel_sb[:], ot_sb[:], start=True, stop=True)
            rec = work.tile([P, QC], F32, tag="rec", name="rec")
            nc.vector.reciprocal_approx_fast(rec[:], bc[:])
            nc.vector.tensor_mul(ot_n[:, g2, :], ot_sb[:], rec[:])
        return tail

    pending_tail = None
    for nn, g2 in steps:
        if g2 == 0:
            ot_n_map[nn] = work.tile([P, 2, QC], F16, tag="otn", name="ot_n")
        ot_n = ot_n_map[nn]
        at = ps_at.tile([P, QC], F32, tag="at", name="at")
        pair = alloc_sc()
        emit_scores(nn, g2, 0, pair, (0, 1, 2, 3))
        pts = {}
        for kt in range(KT):
            cur = pair
            use_dve = dve_mask[kt]
            # exp of the first head-pair (ACT)
            pt01 = work.tile([P, 1024], BF16, tag="pt01", name="pt01")
            nc.scalar.activation(pt01[:], cur[0][:], EXPF)
            # next key-tile's first scores pair (runs under exp(kt))
            if kt + 1 < KT:
                pair = alloc_sc()
                emit_scores(nn, g2, kt + 1, pair, (0, 1))
            # exp of the second head-pair (DVE bit-trick or ACT)
            if use_dve:
                pt23 = work.tile([P, 1024], I16, tag="pt23", name="pt23")
                nc.vector.tensor_scalar(
                    pt23[:], cur[1][:], SCH_C1, SCH_C0, ALU.mult, ALU.add
                )
            else:
                pt23 = work.tile([P, 1024], BF16, tag="pt23", name="pt23")
                nc.scalar.activation(pt23[:], cur[1][:], EXPF)
            pts[kt] = (pt01, pt23, use_dve)
            # deferred tail of the previous step
            if kt == 0 and pending_tail is not None:
                pending_tail()
                pending_tail = None
            # second half of next key-tile's scores BEFORE the attnv batch:
            # the attnv matmuls gated on the (slower) DVE exp must not sit
            # ahead of the next scores in the PE's in-order queue
            if kt + 1 < KT:
                emit_scores(nn, g2, kt + 1, pair, (2, 3))
            # attn@v batched every 4 key-tiles (one PE mode phase)
            if kt % 4 == 3 or kt == KT - 1:
                for ak in sorted(pts):
                    a01, a23, adve = pts[ak]
                    for cj in range(4):
                        src = a01 if cj < 2 else a23
                        ap = src[:, QC * (cj % 2) : QC * (cj % 2 + 1)]
                        if adve and cj >= 2:
                            ap = ap.bitcast(BF16)
                        nc.tensor.matmul(
                            at[32 * cj : 32 * cj + 32, :],
                            vaug[:, ak, 4 * g2 + cj, :],
                            ap,
                            start=(ak == 0),
                            stop=(ak == KT - 1),
                            tile_position=(0, 32 * cj),
                        )
                pts = {}
        pending_tail = make_tail(at, ot_n, nn, g2)
    pending_tail()

    # ---------------- phase C: output projection ----------------
    for nn in range(NQ):
        ot_n = ot_n_map[nn]
        for ss in range(QC // P):
            pp = ps_big.tile([P, C], F32, tag="big", name="pp")
            for gg in range(2):
                nc.tensor.matmul(
                    pp[:],
                    ot_n[:, gg, P * ss : P * (ss + 1)],
                    wp_sb[:, gg, :],
                    start=(gg == 0),
                    stop=(gg == 1),
                )
            ob = work.tile([P, C], F32, tag="ob", name="ob")
            if ss % 2 == 0:
                nc.vector.tensor_copy(ob[:], pp[:])
            else:
                nc.scalar.copy(ob[:], pp[:])
            nc.sync.dma_start(out_r[:, nn * (QC // P) + ss, :], ob[:])


def _get_nc(n_tokens=N_FULL, reps=1):
    key = (n_tokens, DVE_KT, reps, ABLATE)
    if key not in _NC_CACHE:
        _NC_CACHE[key] = build(n_tokens, DVE_KT, reps=reps)
    return _NC_CACHE[key]


def make_core_inputs(core, x, w_qkv, b_qkv, w_proj, n_tokens=N_FULL):
    """Host-side sharding: slice/spread weights for one core."""
    b, g = core // 2, core % 2
    wq_s = np.zeros((2, C, P), np.float32)
    wk_s = np.zeros((2, C, P), np.float32)
    bq_s = np.zeros((2, P), np.float32)
    bk_s = np.zeros((2, P), np.float32)
    wv_s = np.zeros((C, P), np.float32)
    bv_s = np.zeros((P,), np.float32)
    wp_s = np.zeros((2, P, C), np.float32)
    for g2 in range(2):
        for j in range(4):
            h = 8 * g + 4 * g2 + j
            sp = slice(32 * j, 32 * j + D)
            wq_s[g2, :, sp] = w_qkv[:, 0 * C + h * D : 0 * C + (h + 1) * D]
            wk_s[g2, :, sp] = w_qkv[:, 1 * C + h * D : 1 * C + (h + 1) * D]
            bq_s[g2, sp] = b_qkv[0 * C + h * D : 0 * C + (h + 1) * D]
            bk_s[g2, sp] = b_qkv[1 * C + h * D : 1 * C + (h + 1) * D]
            wp_s[g2, sp, :] = w_proj[h * D : (h + 1) * D, :]
    for lh in range(8):
        h = 8 * g + lh
        wv_s[:, 16 * lh : 16 * lh + 16] = w_qkv[:, 2 * C + h * D : 2 * C + (h + 1) * D]
        bv_s[16 * lh : 16 * lh + 16] = b_qkv[2 * C + h * D : 2 * C + (h + 1) * D]
    sel = np.zeros((P, P), np.float32)
    for j in range(4):
        sel[32 * j + 16, 32 * j : 32 * j + 32] = 1.0
    idn = np.eye(P, dtype=np.float32)

    return {
        "x": np.ascontiguousarray(x[b, :n_tokens], dtype=np.float32),
        "wq": wq_s.astype(np.float16),
        "wk": wk_s.astype(np.float16),
        "wv": wv_s.astype(np.float16),
        "bq": bq_s, "bk": bk_s, "bv": bv_s,
        "wp": wp_s.astype(np.float16),
        "sel": sel, "idn": idn,
    }


def kernel(x, w_qkv, b_qkv, w_proj, b_proj):
    global LAST_RESULT
    from concourse.bass_utils import run_bass_kernel_spmd

    x = np.asarray(x, dtype=np.float32)
    w_qkv = np.asarray(w_qkv, dtype=np.float32)
    b_qkv = np.asarray(b_qkv, dtype=np.float32)
    w_proj = np.asarray(w_proj, dtype=np.float32)
    b_proj = np.asarray(b_proj, dtype=np.float32)

    nc = _get_nc(reps=TIMING_REPS)
    in_maps = [
        make_core_inputs(core, x, w_qkv, b_qkv, w_proj) for core in range(NCORES)
    ]
    res = run_bass_kernel_spmd(nc, in_maps, list(range(NCORES)))
    LAST_RESULT = res
    out = np.zeros((B, N_FULL, C), np.float32)
    for core in range(NCORES):
        out[core // 2] += res.results[core]["out"]
    out += b_proj[None, None, :]
    return out
